# revision 6
# baseline (speedup 1.0000x reference)
"""GraphConv + BatchNorm + LeakyReLU fused layer on 8 Trainium2 NeuronCores.

Strategy (dense fp8 edge stream, cell-packed segment sum):
  - Destination nodes are degree-balanced across the 8 cores (snake deal of
    the degree-sorted node list), then packed into "cells" of up to 4 dst
    nodes whose in-edges total <= 128 (fold packing + swap repair). The host
    materializes one dense fp8 (e3m4) stream [128, T, 136] per core in the
    exact SBUF layout: per cell, 128 gathered source-feature edge rows,
    4 one-hot S columns, and 4 fp8 columns of the cell's own features
    (xoT). The device streams it at full dense-DMA bandwidth with zero
    SWDGE gather-descriptor cost.
  - The per-cell segment sum is one PE matmul with the [128 x 4] one-hot S:
    aggT[:, 4c:4c+4] = G_cell^T @ S_cell. Output free dim is 4, so the whole
    aggregation costs ~4 PE cycles per cell; cells' PSUM column windows are
    disjoint so every matmul is start=True/stop=True.
  - Chunks of <=192 cells (one DMA each) split into PSUM-bank groups of
    <=128 cells (512 dst columns): aggT -> bf16 (ACT copy),
    x1T = WrT.T@aggT + WoT.T@xoT (the xoT matmul mixes bf16 weights with
    the fp8 stream operand), one parametric-Lrelu ACT pass gives
    v = leaky(x1 + b_rel), x3T = WlT.T@v, x3 -> bf16 SBUF (DVE copy), and
    per-group BN partial statistics via the native DVE bn_stats (read
    straight from PSUM).
  - Pad dst columns all carry the constant x3 = W_lin @ leaky(b_rel); the
    device computes that constant and subtracts n_pad * c (and n_pad * c^2)
    from the (sum, sumsq) form before the AllReduce, so statistics are
    exact over the 50000 real nodes. bn_aggr folds the per-group stats.
  - BN stats AllReduce via a DRAM bounce (skipped in the single-core
    replica); the final affine + leaky runs in quarters alternating between
    an ACT Lrelu pass (bias/scale/alpha) and a DVE tensor_scalar+max pair,
    each quarter stored immediately (stores alternate Pool/SP queues so
    descriptor-generation latencies overlap). Output is feature-major bf16;
    the host transposes/unpermutes and casts to float32.

kernel(**inputs) takes the full-size numpy inputs and returns the full
[50000, 128] float32 output; everything device-side runs SPMD on cores 0-7.
"""
import sys

if "/opt/trn_rl_repo" not in sys.path:
    sys.path.insert(0, "/opt/trn_rl_repo")

import numpy as np
import ml_dtypes

import concourse.bass as bass
import concourse.mybir as mybir
import concourse.tile as tile
from concourse import bacc
from concourse import bass_utils

F32 = mybir.dt.float32
BF16 = mybir.dt.bfloat16
F8 = mybir.dt.float8e3

N_NODES = 50000
N_CORES = 8
NPC = N_NODES // N_CORES          # 6250 real dst nodes per core
BN_EPS = 1e-5
NEG = 0.01


def _pack_cells(nodes, deg, T, max_iter=4000):
    """Pack `nodes` (approx sorted desc by degree) into T cells of <=4 nodes
    with per-cell degree sum <= 128. Fold packing + swap repair. Returns
    [T, 4] node ids (-1 = empty slot) or None if infeasible."""
    n = len(nodes)
    a = np.full(4 * T, -1, np.int64)
    a[:n] = nodes
    idx = np.arange(T)
    cells = np.stack([a[idx], a[2 * T - 1 - idx], a[2 * T + idx],
                      a[4 * T - 1 - idx]], 1)
    cdeg = np.where(cells >= 0, deg[np.maximum(cells, 0)], 0)
    s = cdeg.sum(1)
    for _ in range(max_iter):
        mx = s.max()
        if mx <= 128:
            return cells
        hi = int(np.argmax(s))
        over = mx - 128
        done = False
        for j in np.argsort(-cdeg[hi]):
            if cells[hi, j] < 0:
                continue
            dj = cdeg[hi, j]
            hi_dk = dj - over
            if hi_dk < 0:
                continue
            lo_dk = np.maximum(s + dj - 128, 0)
            ok = (cdeg >= lo_dk[:, None]) & (cdeg <= hi_dk) & (cells >= 0)
            ok[hi] = False
            tt, kk = np.nonzero(ok)
            if len(tt) == 0:
                continue
            b = int(np.argmax(cdeg[tt, kk]))
            t, k = int(tt[b]), int(kk[b])
            dk = cdeg[t, k]
            cells[hi, j], cells[t, k] = cells[t, k], cells[hi, j]
            cdeg[hi, j], cdeg[t, k] = dk, dj
            s[hi] += dk - dj
            s[t] += dj - dk
            done = True
            break
        if not done:
            return None
    return None


def preprocess(x, edge_index, cfg):
    """Host-side sharding: per-core input dicts (without weights). Sets
    cfg['T'] (cells per core), cfg['n_pad'], and cfg['colmap'] (per-core
    (node ids, device columns) for output unpermutation)."""
    ncores = cfg["n_cores"]
    n = x.shape[0]
    src = np.asarray(edge_index[0], dtype=np.int64)
    dst = np.asarray(edge_index[1], dtype=np.int64)

    deg = np.bincount(dst, minlength=n)
    order = np.argsort(-deg, kind="stable")
    grid = order.reshape(n // ncores, ncores).copy()
    grid[1::2] = grid[1::2, ::-1]  # snake deal: balances per-core edges
    core_nodes = [grid[:, c] for c in range(ncores)]

    T = (n // ncores + 3) // 4
    cells_per_core = None
    while True:
        res = []
        for cn in core_nodes:
            r = _pack_cells(cn, deg, T)
            if r is None:
                # retry with deterministic perturbations of the node order
                for k in (1, 2, 3, 5, 8):
                    cn2 = np.roll(cn, k)
                    r = _pack_cells(cn2, deg, T)
                    if r is not None:
                        break
            res.append(r)
        if all(r is not None for r in res):
            cells_per_core = res
            break
        T += 4
        assert T < 2200, "cell packing runaway"
    cfg["T"] = T
    cfg["n_pad"] = 4 * T - n // ncores

    # node -> (core, cell, pos)
    node_core = np.empty(n, np.int64)
    node_cell = np.empty(n, np.int64)
    node_pos = np.empty(n, np.int64)
    for c in range(ncores):
        cells = cells_per_core[c]
        t_idx, j_idx = np.nonzero(cells >= 0)
        nid = cells[t_idx, j_idx]
        node_core[nid] = c
        node_cell[nid] = t_idx
        node_pos[nid] = j_idx

    # edge -> (core, cell, pos, rank-within-cell)
    ec = node_core[dst]
    et = node_cell[dst]
    ep = node_pos[dst]
    key = ec * T + et
    eorder = np.argsort(key, kind="stable")
    key_s = key[eorder]
    counts = np.bincount(key_s, minlength=ncores * T)
    starts = np.zeros(ncores * T + 1, np.int64)
    np.cumsum(counts, out=starts[1:])
    rank_s = np.arange(len(src)) - starts[key_s]
    assert rank_s.max() < 128
    src_s = src[eorder]
    ec_s = ec[eorder]
    et_s = et[eorder]
    ep_s = ep[eorder]

    xq = x.astype(ml_dtypes.float8_e3m4)

    G_all = np.zeros((ncores, T, 128, 128), dtype=ml_dtypes.float8_e3m4)
    G_all[ec_s, et_s, rank_s] = xq[src_s]
    GS = np.zeros((ncores, 128, T, 136), dtype=ml_dtypes.float8_e3m4)
    GS[ec_s, rank_s, et_s, 128 + ep_s] = 1.0

    per_core = []
    colmap = []
    for c in range(ncores):
        cells = cells_per_core[c]
        t_idx, j_idx = np.nonzero(cells >= 0)
        nid = cells[t_idx, j_idx]
        cols = 4 * t_idx + j_idx
        xoT = np.zeros((128, 4 * T), dtype=ml_dtypes.float8_e3m4)
        xoT[:, cols] = xq[nid].T
        GS[c, :, :, 0:128] = G_all[c].transpose(1, 0, 2)
        GS[c, :, :, 132:136] = xoT.reshape(128, T, 4)
        per_core.append({
            "G": GS[c],
        })
        colmap.append((nid, cols))
    cfg["colmap"] = colmap
    return per_core


def build_program(cfg):
    ncores = cfg["n_cores"]
    T = cfg["T"]
    n_pad = cfg["n_pad"]
    W = 4 * T
    # load chunks (one DMA each); inner groups (<=128 cells = one PSUM
    # bank). First chunk small to shorten the fill; the last chunks taper
    # down so the serial compute chain after the final DMA is short.
    first = cfg.get("first_sz", 32)
    cmax = cfg.get("chunk_max", 192)
    taper = [x for x in cfg.get("taper", (64, 24, 8))]
    sizes = []
    rem = T
    f = min(first, rem)
    sizes.append(f)
    rem -= f
    taper = [x for x in taper if x < rem]
    tail_tot = sum(taper)
    while rem > cmax // 2 + tail_tot:
        take = min(cmax, rem - cmax // 2 - tail_tot)
        sizes.append(take)
        rem -= take
    if rem > tail_tot:
        sizes.append(rem - tail_tot)
        rem = tail_tot
    for x in taper:
        sizes.append(x)
        rem -= x
    if rem:
        sizes.append(rem)
    chunks = []
    c0 = 0
    for sz in sizes:
        chunks.append((c0, sz))
        c0 += sz
    gmax = cfg.get("grp_max", 128)
    groups = []
    for cc0, ccg in chunks:
        g0 = cc0
        while g0 < cc0 + ccg:
            groups.append((g0, min(gmax, cc0 + ccg - g0)))
            g0 += gmax
    ng = len(groups)
    inv_n = 1.0 / float(cfg["n_total"])

    nc = bacc.Bacc("TRN2", target_bir_lowering=False, debug=False,
                   num_devices=ncores)

    G_d = nc.dram_tensor("G", [128, T, 136], F8, kind="ExternalInput")
    wc_d = nc.dram_tensor("Wcat", [128, 384], BF16, kind="ExternalInput")
    pc_d = nc.dram_tensor("Pcat", [128, 3], F32, kind="ExternalInput")
    out_d = nc.dram_tensor("out", [128, W], BF16, kind="ExternalOutput")

    AF = mybir.ActivationFunctionType
    with tile.TileContext(nc) as tc:
        with (
            tc.tile_pool(name="gp", bufs=cfg.get("gp_bufs", 3)) as gp,
            tc.tile_pool(name="ps", bufs=cfg.get("ps_bufs", 6), space="PSUM") as ps,
            tc.tile_pool(name="misc", bufs=cfg.get("misc_bufs", 3)) as misc,
            tc.tile_pool(name="big", bufs=1) as big,
            tc.tile_pool(name="dram", bufs=1, space="DRAM") as dram,
        ):
            consts = big
            wcat = consts.tile([128, 384], BF16)
            pcat = consts.tile([128, 3], F32)
            x3_s = big.tile([128, W], BF16)
            out_sb = big.tile([128, W], BF16)
            bns = big.tile([128, ng, 6], F32)

            cc0_0, ccg_0 = chunks[0]
            Gt0 = gp.tile([128, cmax, 136], F8, tag="G")
            nc.sync.dma_start(Gt0[:, 0:ccg_0, :],
                              G_d[:, cc0_0:cc0_0 + ccg_0, :])
            nc.sync.dma_start(wcat[:], wc_d[:])
            nc.sync.dma_start(pcat[:], pc_d[:])

            # c* = W_lin @ leaky(b_rel): the x3 value of every pad column.
            zero1 = consts.tile([128, 1], F32)
            nc.vector.memset(zero1[:], 0.0)
            epsv = consts.tile([128, 1], F32)
            nc.vector.memset(epsv[:], BN_EPS)
            # Warm-up Sqrt: the act-table pass loads the set containing Sqrt
            # first; Prelu (used for every leaky pass) is in the same set, so
            # no further 1.28us table load lands on the critical tail.
            warm = consts.tile([128, 1], F32)
            nc.scalar.activation(warm[:], zero1[:], AF.Sqrt, bias=epsv[:],
                                 scale=1.0)
            vb = consts.tile([128, 1], BF16)
            nc.scalar.activation(vb[:], pcat[:, 0:1], AF.Prelu, bias=zero1[:],
                                 scale=1.0, alpha=NEG)
            cst_ps = ps.tile([128, 1], F32, tag="ps")
            nc.tensor.matmul(cst_ps[:], lhsT=wcat[:, 256:384], rhs=vb[:],
                             start=True, stop=True)
            cst = consts.tile([128, 1], F32)
            cst2 = consts.tile([128, 1], F32)
            cpn = consts.tile([128, 1], F32)
            cp2n = consts.tile([128, 1], F32)
            nc.scalar.copy(cst[:], cst_ps[:])
            nc.vector.tensor_tensor(out=cst2[:], in0=cst[:], in1=cst[:],
                                    op=mybir.AluOpType.mult)
            pad_frac = float(n_pad) / float(cfg["n_total"])
            nc.vector.tensor_scalar_mul(cpn[:], cst[:], pad_frac)
            nc.vector.tensor_scalar_mul(cp2n[:], cst2[:], pad_frac)

            gidx = 0
            for ci, (cc0, ccg) in enumerate(chunks):
                if ci == 0:
                    Gt = Gt0
                else:
                    Gt = gp.tile([128, cmax, 136], F8, tag="G")
                    nc.gpsimd.dma_start(Gt[:, 0:ccg, :],
                                        G_d[:, cc0:cc0 + ccg, :])
                g0 = cc0
                while g0 < cc0 + ccg:
                    cg = min(gmax, cc0 + ccg - g0)
                    goff = g0 - cc0
                    g = gidx
                    gidx += 1
                    agg_ps = ps.tile([128, 128, 4], F32, tag="ps")
                    for i in range(cg):
                        nc.tensor.matmul(agg_ps[:, i, :],
                                         lhsT=Gt[:, goff + i, 0:128],
                                         rhs=Gt[:, goff + i, 128:132],
                                         start=True, stop=True)
                    aggs = misc.tile([128, 512], BF16, tag="aggs")
                    nc.scalar.copy(aggs[:, 0:cg * 4], agg_ps[:, 0:cg, :])

                    x1_ps = ps.tile([128, 512], F32, tag="ps")
                    nc.tensor.matmul(x1_ps[:, 0:cg * 4], lhsT=wcat[:, 0:128],
                                     rhs=aggs[:, 0:cg * 4], start=True,
                                     stop=False)
                    nc.tensor.matmul(x1_ps[:, 0:cg * 4], lhsT=wcat[:, 128:256],
                                     rhs=Gt[:, goff:goff + cg, 132:136],
                                     start=False, stop=True)
                    v_t = misc.tile([128, 512], BF16, tag="v")
                    nc.scalar.activation(v_t[:, 0:cg * 4], x1_ps[:, 0:cg * 4],
                                         AF.Prelu, bias=pcat[:, 0:1], scale=1.0,
                                         alpha=NEG)
                    x3_ps = ps.tile([128, 512], F32, tag="ps")
                    nc.tensor.matmul(x3_ps[:, 0:cg * 4], lhsT=wcat[:, 256:384],
                                     rhs=v_t[:, 0:cg * 4], start=True,
                                     stop=True)
                    xr = x3_s[:, 4 * g0:4 * (g0 + cg)]
                    nc.vector.bn_stats(bns[:, g, :], x3_ps[:, 0:cg * 4])
                    nc.vector.tensor_copy(xr, x3_ps[:, 0:cg * 4])
                    g0 += gmax

            # ---- global BN statistics (pad-corrected) via AllReduce ----
            mv = consts.tile([128, 2], F32)
            nc.vector.bn_aggr(mv[:], bns[:])
            if ncores > 1 and not cfg.get("no_cc"):
                sumt = consts.tile([128, 1], F32)
                sqt = consts.tile([128, 1], F32)
                stat2 = consts.tile([128, 2], F32)
                # sum = W*mean ; sumsq = W*(var + mean^2), minus pad terms
                nc.vector.tensor_scalar(out=sumt[:], in0=mv[:, 0:1],
                                        scalar1=float(W), scalar2=None,
                                        op0=mybir.AluOpType.mult)
                nc.vector.scalar_tensor_tensor(
                    out=sqt[:], in0=mv[:, 0:1], scalar=mv[:, 0:1],
                    in1=mv[:, 1:2], op0=mybir.AluOpType.mult,
                    op1=mybir.AluOpType.add)
                nc.vector.tensor_scalar(out=sqt[:], in0=sqt[:],
                                        scalar1=float(W), scalar2=None,
                                        op0=mybir.AluOpType.mult)
                nc.vector.scalar_tensor_tensor(
                    out=stat2[:, 0:1], in0=cst[:], scalar=-float(n_pad),
                    in1=sumt[:], op0=mybir.AluOpType.mult,
                    op1=mybir.AluOpType.add)
                nc.vector.scalar_tensor_tensor(
                    out=stat2[:, 1:2], in0=cst2[:], scalar=-float(n_pad),
                    in1=sqt[:], op0=mybir.AluOpType.mult,
                    op1=mybir.AluOpType.add)
                cc_in = dram.tile([128, 2], F32)
                cc_out = dram.tile([128, 2], F32)
                nc.sync.dma_start(cc_in[:], stat2[:])
                nc.gpsimd.collective_compute(
                    "AllReduce",
                    mybir.AluOpType.add,
                    replica_groups=[list(range(ncores))],
                    ins=[cc_in[:].opt()],
                    outs=[cc_out[:].opt()],
                )
                stat_r = consts.tile([128, 2], F32)
                nc.sync.dma_start(stat_r[:], cc_out[:])
            me2n = consts.tile([128, 2], F32)
            nvar = consts.tile([128, 1], F32)
            rstd = consts.tile([128, 1], F32)
            scl = consts.tile([128, 1], F32)
            bia = consts.tile([128, 1], F32)
            if ncores > 1 and not cfg.get("no_cc"):
                # me2n = [-mean, -E[x^2]] over the real nodes
                nc.vector.tensor_scalar_mul(me2n[:], stat_r[:], -inv_n)
            else:
                # folded: -mean = mv0*(-W/n) + c*(n_pad/n), similarly E[x^2]
                wn = -float(W) * inv_n
                nc.vector.scalar_tensor_tensor(
                    out=me2n[:, 0:1], in0=mv[:, 0:1], scalar=wn,
                    in1=cpn[:], op0=mybir.AluOpType.mult,
                    op1=mybir.AluOpType.add)
                ex2p = consts.tile([128, 1], F32)
                nc.vector.scalar_tensor_tensor(
                    out=ex2p[:], in0=mv[:, 0:1], scalar=mv[:, 0:1],
                    in1=mv[:, 1:2], op0=mybir.AluOpType.mult,
                    op1=mybir.AluOpType.add)
                nc.vector.scalar_tensor_tensor(
                    out=me2n[:, 1:2], in0=ex2p[:], scalar=wn,
                    in1=cp2n[:], op0=mybir.AluOpType.mult,
                    op1=mybir.AluOpType.add)
            # nvar = mean^2 - E[x^2] = -var
            nc.vector.scalar_tensor_tensor(
                out=nvar[:], in0=me2n[:, 0:1], scalar=me2n[:, 0:1],
                in1=me2n[:, 1:2], op0=mybir.AluOpType.mult,
                op1=mybir.AluOpType.add)
            # rstd = 1/sqrt(var + eps) = 1/sqrt(-nvar + eps)
            nc.scalar.activation(rstd[:], nvar[:], AF.Sqrt, bias=epsv[:],
                                 scale=-1.0)
            nc.vector.reciprocal(rstd[:], rstd[:])
            nc.vector.tensor_tensor(out=scl[:], in0=pcat[:, 1:2], in1=rstd[:],
                                    op=mybir.AluOpType.mult)
            # bia = beta - mean*scl = (-mean)*scl + beta
            nc.vector.scalar_tensor_tensor(
                out=bia[:], in0=me2n[:, 0:1], scalar=scl[:], in1=pcat[:, 2:3],
                op0=mybir.AluOpType.mult, op1=mybir.AluOpType.add)

            # ---- normalize + leaky + store ----
            # Pieces run on ACT (one fused Prelu pass), DVE (fused affine +
            # mult/max pass), and Pool (same two passes, slower) in parallel;
            # every store's descriptor-gen goes on the otherwise-idle SP
            # HWDGE queue so no compute engine stalls on a gen.
            pieces = cfg.get("tail_pieces", (
                ("a", 0.055), ("d", 0.045),
                ("a", 0.105), ("d", 0.085),
                ("a", 0.155), ("d", 0.12),
                ("a", 0.19), ("d", 0.145),
                ("a", 0.10),
        ))
            tot = sum(f for _, f in pieces)
            h0 = 0
            for i, (eng_c, f) in enumerate(pieces):
                h1 = W if i == len(pieces) - 1 else min(
                    W, h0 + max(256, (int(W * f / tot) + 7) & ~7))
                if h1 <= h0:
                    continue
                if eng_c == "a":
                    nc.scalar.activation(out_sb[:, h0:h1], x3_s[:, h0:h1],
                                         AF.Prelu, bias=bia[:], scale=scl[:],
                                         alpha=NEG)
                else:
                    v_eng = nc.vector if eng_c == "d" else nc.gpsimd
                    xq_ap = x3_s[:, h0:h1]
                    v_eng.tensor_scalar(
                        out=xq_ap, in0=xq_ap, scalar1=scl[:], scalar2=bia[:],
                        op0=mybir.AluOpType.mult, op1=mybir.AluOpType.add)
                    v_eng.scalar_tensor_tensor(
                        out=out_sb[:, h0:h1], in0=xq_ap, scalar=NEG,
                        in1=xq_ap, op0=mybir.AluOpType.mult,
                        op1=mybir.AluOpType.max)
                nc.sync.dma_start(out_d[:, h0:h1], out_sb[:, h0:h1])
                h0 = h1

    nc.compile()
    return nc


_PROGRAM_CACHE = {}


def run(x, edge_index, W_rel, b_rel, W_root, W_lin, b_lin, gamma, beta, cfg):
    per_core = preprocess(x, edge_index, cfg)

    wcat = np.concatenate([W_rel.T, W_root.T, W_lin.T], axis=1)
    pcat = np.stack([b_rel, gamma, beta], axis=1)
    shared = {
        "Wcat": np.ascontiguousarray(wcat).astype(ml_dtypes.bfloat16),
        "Pcat": np.ascontiguousarray(pcat).astype(np.float32),
    }
    # b_lin is dropped: it shifts every x3 column equally, so BatchNorm's
    # mean subtraction cancels it exactly.
    in_maps = [dict(m, **shared) for m in per_core]

    key = (cfg["n_cores"], cfg["T"])
    if key not in _PROGRAM_CACHE:
        _PROGRAM_CACHE[key] = build_program(cfg)
    nc = _PROGRAM_CACHE[key]

    res = bass_utils.run_bass_kernel_spmd(
        nc, in_maps, core_ids=list(range(cfg["n_cores"])))
    n = x.shape[0]
    out = np.empty((n, 128), dtype=np.float32)
    for c in range(cfg["n_cores"]):
        nid, cols = cfg["colmap"][c]
        dev = np.asarray(res.results[c]["out"])  # [128, 4T] bf16
        out[nid] = dev[:, cols].T.astype(np.float32)
    return out


def make_cfg():
    return {
        "n_cores": N_CORES,
        "npc": NPC,
        "n_total": N_NODES,
    }


def kernel(x, edge_index, batch, W_rel, b_rel, W_root, W_lin, b_lin, gamma,
           beta):
    x = np.asarray(x, dtype=np.float32)
    cfg = make_cfg()
    return run(x, np.asarray(edge_index), np.asarray(W_rel, dtype=np.float32),
               np.asarray(b_rel, dtype=np.float32),
               np.asarray(W_root, dtype=np.float32),
               np.asarray(W_lin, dtype=np.float32),
               np.asarray(b_lin, dtype=np.float32),
               np.asarray(gamma, dtype=np.float32),
               np.asarray(beta, dtype=np.float32), cfg)



# revision 10
# speedup vs baseline: 1.0097x; 1.0097x over previous
"""GraphConv + BatchNorm + LeakyReLU fused layer on 8 Trainium2 NeuronCores.

Strategy (dense fp8 edge stream, cell-packed segment sum):
  - Destination nodes are degree-balanced across the 8 cores (snake deal of
    the degree-sorted node list), then packed into "cells" of up to 4 dst
    nodes whose in-edges total <= 128 (fold packing + swap repair). The host
    materializes one dense fp8 (e3m4) stream [128, T, 136] per core in the
    exact SBUF layout: per cell, 128 gathered source-feature edge rows,
    4 one-hot S columns, and 4 fp8 columns of the cell's own features
    (xoT). The device streams it at full dense-DMA bandwidth with zero
    SWDGE gather-descriptor cost.
  - The per-cell segment sum is one PE matmul with the [128 x 4] one-hot S:
    aggT[:, 4c:4c+4] = G_cell^T @ S_cell. Output free dim is 4, so the whole
    aggregation costs ~4 PE cycles per cell; cells' PSUM column windows are
    disjoint so every matmul is start=True/stop=True.
  - Chunks of <=192 cells (one DMA each) split into PSUM-bank groups of
    <=128 cells (512 dst columns): aggT -> bf16 (ACT copy),
    x1T = WrT.T@aggT + WoT.T@xoT (the xoT matmul mixes bf16 weights with
    the fp8 stream operand), one parametric-Lrelu ACT pass gives
    v = leaky(x1 + b_rel), x3T = WlT.T@v, x3 -> bf16 SBUF (DVE copy), and
    per-group BN partial statistics via the native DVE bn_stats (read
    straight from PSUM).
  - Pad dst columns all carry the constant x3 = W_lin @ leaky(b_rel); the
    device computes that constant and subtracts n_pad * c (and n_pad * c^2)
    from the (sum, sumsq) form before the AllReduce, so statistics are
    exact over the 50000 real nodes. bn_aggr folds the per-group stats.
  - BN stats AllReduce via a DRAM bounce (skipped in the single-core
    replica); the final affine + leaky runs in quarters alternating between
    an ACT Lrelu pass (bias/scale/alpha) and a DVE tensor_scalar+max pair,
    each quarter stored immediately (stores alternate Pool/SP queues so
    descriptor-generation latencies overlap). Output is feature-major bf16;
    the host transposes/unpermutes and casts to float32.

kernel(**inputs) takes the full-size numpy inputs and returns the full
[50000, 128] float32 output; everything device-side runs SPMD on cores 0-7.
"""
import sys

if "/opt/trn_rl_repo" not in sys.path:
    sys.path.insert(0, "/opt/trn_rl_repo")

import numpy as np
import ml_dtypes

import concourse.bass as bass
import concourse.mybir as mybir
import concourse.tile as tile
from concourse import bacc
from concourse import bass_utils

F32 = mybir.dt.float32
BF16 = mybir.dt.bfloat16
F8 = mybir.dt.float8e3

N_NODES = 50000
N_CORES = 8
NPC = N_NODES // N_CORES          # 6250 real dst nodes per core
BN_EPS = 1e-5
NEG = 0.01


def _pack_cells(nodes, deg, T, max_iter=4000):
    """Pack `nodes` (approx sorted desc by degree) into T cells of <=4 nodes
    with per-cell degree sum <= 128. Fold packing + swap repair. Returns
    [T, 4] node ids (-1 = empty slot) or None if infeasible."""
    n = len(nodes)
    a = np.full(4 * T, -1, np.int64)
    a[:n] = nodes
    idx = np.arange(T)
    cells = np.stack([a[idx], a[2 * T - 1 - idx], a[2 * T + idx],
                      a[4 * T - 1 - idx]], 1)
    cdeg = np.where(cells >= 0, deg[np.maximum(cells, 0)], 0)
    s = cdeg.sum(1)
    for _ in range(max_iter):
        mx = s.max()
        if mx <= 128:
            return cells
        hi = int(np.argmax(s))
        over = mx - 128
        done = False
        for j in np.argsort(-cdeg[hi]):
            if cells[hi, j] < 0:
                continue
            dj = cdeg[hi, j]
            hi_dk = dj - over
            if hi_dk < 0:
                continue
            lo_dk = np.maximum(s + dj - 128, 0)
            ok = (cdeg >= lo_dk[:, None]) & (cdeg <= hi_dk) & (cells >= 0)
            ok[hi] = False
            tt, kk = np.nonzero(ok)
            if len(tt) == 0:
                continue
            b = int(np.argmax(cdeg[tt, kk]))
            t, k = int(tt[b]), int(kk[b])
            dk = cdeg[t, k]
            cells[hi, j], cells[t, k] = cells[t, k], cells[hi, j]
            cdeg[hi, j], cdeg[t, k] = dk, dj
            s[hi] += dk - dj
            s[t] += dj - dk
            done = True
            break
        if not done:
            return None
    return None


def preprocess(x, edge_index, cfg):
    """Host-side sharding: per-core input dicts (without weights). Sets
    cfg['T'] (cells per core), cfg['n_pad'], and cfg['colmap'] (per-core
    (node ids, device columns) for output unpermutation)."""
    ncores = cfg["n_cores"]
    n = x.shape[0]
    src = np.asarray(edge_index[0], dtype=np.int64)
    dst = np.asarray(edge_index[1], dtype=np.int64)

    deg = np.bincount(dst, minlength=n)
    order = np.argsort(-deg, kind="stable")
    grid = order.reshape(n // ncores, ncores).copy()
    grid[1::2] = grid[1::2, ::-1]  # snake deal: balances per-core edges
    core_nodes = [grid[:, c] for c in range(ncores)]

    T = (n // ncores + 3) // 4
    cells_per_core = None
    while True:
        res = []
        for cn in core_nodes:
            r = _pack_cells(cn, deg, T)
            if r is None:
                # retry with deterministic perturbations of the node order
                for k in (1, 2, 3, 5, 8):
                    cn2 = np.roll(cn, k)
                    r = _pack_cells(cn2, deg, T)
                    if r is not None:
                        break
            res.append(r)
        if all(r is not None for r in res):
            cells_per_core = res
            break
        T += 4
        assert T < 2200, "cell packing runaway"
    cfg["T"] = T
    cfg["n_pad"] = 4 * T - n // ncores

    # node -> (core, cell, pos)
    node_core = np.empty(n, np.int64)
    node_cell = np.empty(n, np.int64)
    node_pos = np.empty(n, np.int64)
    for c in range(ncores):
        cells = cells_per_core[c]
        t_idx, j_idx = np.nonzero(cells >= 0)
        nid = cells[t_idx, j_idx]
        node_core[nid] = c
        node_cell[nid] = t_idx
        node_pos[nid] = j_idx

    # edge -> (core, cell, pos, rank-within-cell)
    ec = node_core[dst]
    et = node_cell[dst]
    ep = node_pos[dst]
    key = ec * T + et
    eorder = np.argsort(key, kind="stable")
    key_s = key[eorder]
    counts = np.bincount(key_s, minlength=ncores * T)
    starts = np.zeros(ncores * T + 1, np.int64)
    np.cumsum(counts, out=starts[1:])
    rank_s = np.arange(len(src)) - starts[key_s]
    assert rank_s.max() < 128
    src_s = src[eorder]
    ec_s = ec[eorder]
    et_s = et[eorder]
    ep_s = ep[eorder]

    xq = x.astype(ml_dtypes.float8_e3m4)

    G_all = np.zeros((ncores, T, 128, 128), dtype=ml_dtypes.float8_e3m4)
    G_all[ec_s, et_s, rank_s] = xq[src_s]
    GS = np.zeros((ncores, 128, T, 136), dtype=ml_dtypes.float8_e3m4)
    GS[ec_s, rank_s, et_s, 128 + ep_s] = 1.0

    per_core = []
    colmap = []
    for c in range(ncores):
        cells = cells_per_core[c]
        t_idx, j_idx = np.nonzero(cells >= 0)
        nid = cells[t_idx, j_idx]
        cols = 4 * t_idx + j_idx
        xoT = np.zeros((128, 4 * T), dtype=ml_dtypes.float8_e3m4)
        xoT[:, cols] = xq[nid].T
        GS[c, :, :, 0:128] = G_all[c].transpose(1, 0, 2)
        GS[c, :, :, 132:136] = xoT.reshape(128, T, 4)
        per_core.append({
            "G": GS[c],
        })
        colmap.append((nid, cols))
    cfg["colmap"] = colmap
    return per_core


def build_program(cfg):
    ncores = cfg["n_cores"]
    T = cfg["T"]
    n_pad = cfg["n_pad"]
    W = 4 * T
    # load chunks (one DMA each); inner groups (<=128 cells = one PSUM
    # bank). First chunk small to shorten the fill; the last chunks taper
    # down so the serial compute chain after the final DMA is short.
    first = cfg.get("first_sz", 32)
    cmax = cfg.get("chunk_max", 192)
    taper = [x for x in cfg.get("taper", (32,))]
    sizes = []
    rem = T
    f = min(first, rem)
    sizes.append(f)
    rem -= f
    taper = [x for x in taper if x < rem]
    tail_tot = sum(taper)
    while rem > cmax // 2 + tail_tot:
        take = min(cmax, rem - cmax // 2 - tail_tot)
        sizes.append(take)
        rem -= take
    if rem > tail_tot:
        sizes.append(rem - tail_tot)
        rem = tail_tot
    for x in taper:
        sizes.append(x)
        rem -= x
    if rem:
        sizes.append(rem)
    chunks = []
    c0 = 0
    for sz in sizes:
        chunks.append((c0, sz))
        c0 += sz
    gmax = cfg.get("grp_max", 128)
    groups = []
    for cc0, ccg in chunks:
        g0 = cc0
        while g0 < cc0 + ccg:
            groups.append((g0, min(gmax, cc0 + ccg - g0)))
            g0 += gmax
    ng = len(groups)
    inv_n = 1.0 / float(cfg["n_total"])

    nc = bacc.Bacc("TRN2", target_bir_lowering=False, debug=False,
                   num_devices=ncores)

    G_d = nc.dram_tensor("G", [128, T, 136], F8, kind="ExternalInput")
    wc_d = nc.dram_tensor("Wcat", [128, 384], BF16, kind="ExternalInput")
    pc_d = nc.dram_tensor("Pcat", [128, 3], F32, kind="ExternalInput")
    out_d = nc.dram_tensor("out", [128, W], BF16, kind="ExternalOutput")

    AF = mybir.ActivationFunctionType
    with tile.TileContext(nc) as tc:
        with (
            tc.tile_pool(name="gp", bufs=cfg.get("gp_bufs", 3)) as gp,
            tc.tile_pool(name="tp", bufs=2) as tp,
            tc.tile_pool(name="ps", bufs=cfg.get("ps_bufs", 6), space="PSUM") as ps,
            tc.tile_pool(name="misc", bufs=cfg.get("misc_bufs", 3)) as misc,
            tc.tile_pool(name="big", bufs=1) as big,
            tc.tile_pool(name="dram", bufs=1, space="DRAM") as dram,
        ):
            consts = big
            wcat = consts.tile([128, 384], BF16)
            pcat = consts.tile([128, 3], F32)
            x3_s = big.tile([128, W], BF16)
            out_sb = big.tile([128, W], BF16)
            bns = big.tile([128, ng, 6], F32)

            cc0_0, ccg_0 = chunks[0]
            Gt0 = gp.tile([128, cmax, 136], F8, tag="G")
            nc.sync.dma_start(Gt0[:, 0:ccg_0, :],
                              G_d[:, cc0_0:cc0_0 + ccg_0, :])
            nc.sync.dma_start(wcat[:], wc_d[:])
            nc.sync.dma_start(pcat[:], pc_d[:])

            # c* = W_lin @ leaky(b_rel): the x3 value of every pad column.
            zero1 = consts.tile([128, 1], F32)
            nc.vector.memset(zero1[:], 0.0)
            epsv = consts.tile([128, 1], F32)
            nc.vector.memset(epsv[:], BN_EPS)
            # Warm-up Sqrt: the act-table pass loads the set containing Sqrt
            # first; Prelu (used for every leaky pass) is in the same set, so
            # no further 1.28us table load lands on the critical tail.
            warm = consts.tile([128, 1], F32)
            nc.scalar.activation(warm[:], zero1[:], AF.Sqrt, bias=epsv[:],
                                 scale=1.0)
            vb = consts.tile([128, 1], BF16)
            nc.scalar.activation(vb[:], pcat[:, 0:1], AF.Prelu, bias=zero1[:],
                                 scale=1.0, alpha=NEG)
            cst_ps = ps.tile([128, 1], F32, tag="ps")
            nc.tensor.matmul(cst_ps[:], lhsT=wcat[:, 256:384], rhs=vb[:],
                             start=True, stop=True)
            cst = consts.tile([128, 1], F32)
            cst2 = consts.tile([128, 1], F32)
            cpn = consts.tile([128, 1], F32)
            cp2n = consts.tile([128, 1], F32)
            nc.scalar.copy(cst[:], cst_ps[:])
            nc.vector.tensor_tensor(out=cst2[:], in0=cst[:], in1=cst[:],
                                    op=mybir.AluOpType.mult)
            pad_frac = float(n_pad) / float(cfg["n_total"])
            nc.vector.tensor_scalar_mul(cpn[:], cst[:], pad_frac)
            nc.vector.tensor_scalar_mul(cp2n[:], cst2[:], pad_frac)

            gidx = 0
            n_taper = len(taper)
            for ci, (cc0, ccg) in enumerate(chunks):
                if ci == 0:
                    Gt = Gt0
                elif ci >= len(chunks) - n_taper:
                    # small taper chunks draw from their own pool so their
                    # DMA never waits on a big-chunk buffer being recycled
                    Gt = tp.tile([128, ccg, 136], F8, tag=f"T{ccg}")
                    nc.gpsimd.dma_start(Gt[:, 0:ccg, :],
                                        G_d[:, cc0:cc0 + ccg, :])
                else:
                    Gt = gp.tile([128, cmax, 136], F8, tag="G")
                    nc.gpsimd.dma_start(Gt[:, 0:ccg, :],
                                        G_d[:, cc0:cc0 + ccg, :])
                g0 = cc0
                while g0 < cc0 + ccg:
                    cg = min(gmax, cc0 + ccg - g0)
                    goff = g0 - cc0
                    g = gidx
                    gidx += 1
                    agg_ps = ps.tile([128, 128, 4], F32, tag="ps")
                    for i in range(cg):
                        nc.tensor.matmul(agg_ps[:, i, :],
                                         lhsT=Gt[:, goff + i, 0:128],
                                         rhs=Gt[:, goff + i, 128:132],
                                         start=True, stop=True)
                    aggs = misc.tile([128, 512], BF16, tag="aggs")
                    nc.scalar.copy(aggs[:, 0:cg * 4], agg_ps[:, 0:cg, :])

                    x1_ps = ps.tile([128, 512], F32, tag="ps")
                    nc.tensor.matmul(x1_ps[:, 0:cg * 4], lhsT=wcat[:, 0:128],
                                     rhs=aggs[:, 0:cg * 4], start=True,
                                     stop=False)
                    nc.tensor.matmul(x1_ps[:, 0:cg * 4], lhsT=wcat[:, 128:256],
                                     rhs=Gt[:, goff:goff + cg, 132:136],
                                     start=False, stop=True)
                    v_t = misc.tile([128, 512], BF16, tag="v")
                    nc.scalar.activation(v_t[:, 0:cg * 4], x1_ps[:, 0:cg * 4],
                                         AF.Prelu, bias=pcat[:, 0:1], scale=1.0,
                                         alpha=NEG)
                    x3_ps = ps.tile([128, 512], F32, tag="ps")
                    nc.tensor.matmul(x3_ps[:, 0:cg * 4], lhsT=wcat[:, 256:384],
                                     rhs=v_t[:, 0:cg * 4], start=True,
                                     stop=True)
                    xr = x3_s[:, 4 * g0:4 * (g0 + cg)]
                    nc.vector.bn_stats(bns[:, g, :], x3_ps[:, 0:cg * 4])
                    nc.vector.tensor_copy(xr, x3_ps[:, 0:cg * 4])
                    g0 += gmax

            # ---- global BN statistics (pad-corrected) via AllReduce ----
            mv = consts.tile([128, 2], F32)
            nc.vector.bn_aggr(mv[:], bns[:])
            if ncores > 1 and not cfg.get("no_cc"):
                sumt = consts.tile([128, 1], F32)
                sqt = consts.tile([128, 1], F32)
                stat2 = consts.tile([128, 2], F32)
                # sum = W*mean ; sumsq = W*(var + mean^2), minus pad terms
                nc.vector.tensor_scalar(out=sumt[:], in0=mv[:, 0:1],
                                        scalar1=float(W), scalar2=None,
                                        op0=mybir.AluOpType.mult)
                nc.vector.scalar_tensor_tensor(
                    out=sqt[:], in0=mv[:, 0:1], scalar=mv[:, 0:1],
                    in1=mv[:, 1:2], op0=mybir.AluOpType.mult,
                    op1=mybir.AluOpType.add)
                nc.vector.tensor_scalar(out=sqt[:], in0=sqt[:],
                                        scalar1=float(W), scalar2=None,
                                        op0=mybir.AluOpType.mult)
                nc.vector.scalar_tensor_tensor(
                    out=stat2[:, 0:1], in0=cst[:], scalar=-float(n_pad),
                    in1=sumt[:], op0=mybir.AluOpType.mult,
                    op1=mybir.AluOpType.add)
                nc.vector.scalar_tensor_tensor(
                    out=stat2[:, 1:2], in0=cst2[:], scalar=-float(n_pad),
                    in1=sqt[:], op0=mybir.AluOpType.mult,
                    op1=mybir.AluOpType.add)
                cc_in = dram.tile([128, 2], F32)
                cc_out = dram.tile([128, 2], F32)
                nc.sync.dma_start(cc_in[:], stat2[:])
                nc.gpsimd.collective_compute(
                    "AllReduce",
                    mybir.AluOpType.add,
                    replica_groups=[list(range(ncores))],
                    ins=[cc_in[:].opt()],
                    outs=[cc_out[:].opt()],
                )
                stat_r = consts.tile([128, 2], F32)
                nc.sync.dma_start(stat_r[:], cc_out[:])
            me2n = consts.tile([128, 2], F32)
            nvar = consts.tile([128, 1], F32)
            rstd = consts.tile([128, 1], F32)
            scl = consts.tile([128, 1], F32)
            bia = consts.tile([128, 1], F32)
            if ncores > 1 and not cfg.get("no_cc"):
                # me2n = [-mean, -E[x^2]] over the real nodes
                nc.vector.tensor_scalar_mul(me2n[:], stat_r[:], -inv_n)
            else:
                # folded: -mean = mv0*(-W/n) + c*(n_pad/n), similarly E[x^2]
                wn = -float(W) * inv_n
                nc.vector.scalar_tensor_tensor(
                    out=me2n[:, 0:1], in0=mv[:, 0:1], scalar=wn,
                    in1=cpn[:], op0=mybir.AluOpType.mult,
                    op1=mybir.AluOpType.add)
                ex2p = consts.tile([128, 1], F32)
                nc.vector.scalar_tensor_tensor(
                    out=ex2p[:], in0=mv[:, 0:1], scalar=mv[:, 0:1],
                    in1=mv[:, 1:2], op0=mybir.AluOpType.mult,
                    op1=mybir.AluOpType.add)
                nc.vector.scalar_tensor_tensor(
                    out=me2n[:, 1:2], in0=ex2p[:], scalar=wn,
                    in1=cp2n[:], op0=mybir.AluOpType.mult,
                    op1=mybir.AluOpType.add)
            # nvar = mean^2 - E[x^2] = -var
            nc.vector.scalar_tensor_tensor(
                out=nvar[:], in0=me2n[:, 0:1], scalar=me2n[:, 0:1],
                in1=me2n[:, 1:2], op0=mybir.AluOpType.mult,
                op1=mybir.AluOpType.add)
            # rstd = 1/sqrt(var + eps) = 1/sqrt(-nvar + eps)
            nc.scalar.activation(rstd[:], nvar[:], AF.Sqrt, bias=epsv[:],
                                 scale=-1.0)
            nc.vector.reciprocal(rstd[:], rstd[:])
            nc.vector.tensor_tensor(out=scl[:], in0=pcat[:, 1:2], in1=rstd[:],
                                    op=mybir.AluOpType.mult)
            # bia = beta - mean*scl = (-mean)*scl + beta
            nc.vector.scalar_tensor_tensor(
                out=bia[:], in0=me2n[:, 0:1], scalar=scl[:], in1=pcat[:, 2:3],
                op0=mybir.AluOpType.mult, op1=mybir.AluOpType.add)

            # ---- normalize + leaky + store ----
            # Pieces run on ACT (one fused Prelu pass), DVE (fused affine +
            # mult/max pass), and Pool (same two passes, slower) in parallel;
            # every store's descriptor-gen goes on the otherwise-idle SP
            # HWDGE queue so no compute engine stalls on a gen.
            pieces = cfg.get("tail_pieces", (
                ("a", 0.05), ("d", 0.08),
                ("a", 0.15), ("d", 0.16),
                ("a", 0.17), ("d", 0.19),
                ("a", 0.20),
        ))
            tot = sum(f for _, f in pieces)
            h0 = 0
            for i, (eng_c, f) in enumerate(pieces):
                h1 = W if i == len(pieces) - 1 else min(
                    W, h0 + max(256, (int(W * f / tot) + 7) & ~7))
                if h1 <= h0:
                    continue
                if eng_c == "a":
                    nc.scalar.activation(out_sb[:, h0:h1], x3_s[:, h0:h1],
                                         AF.Prelu, bias=bia[:], scale=scl[:],
                                         alpha=NEG)
                else:
                    xq_ap = x3_s[:, h0:h1]
                    nc.vector.tensor_scalar(
                        out=xq_ap, in0=xq_ap, scalar1=scl[:], scalar2=bia[:],
                        op0=mybir.AluOpType.mult, op1=mybir.AluOpType.add)
                    nc.vector.scalar_tensor_tensor(
                        out=out_sb[:, h0:h1], in0=xq_ap, scalar=NEG,
                        in1=xq_ap, op0=mybir.AluOpType.mult,
                        op1=mybir.AluOpType.max)
                # alternate descriptor-gen queues (SP HWDGE / Pool SWDGE) so
                # gen latency pipelines ahead of the store transfers
                eng = nc.sync if i % 2 == 0 else nc.gpsimd
                eng.dma_start(out_d[:, h0:h1], out_sb[:, h0:h1])
                h0 = h1

    nc.compile()
    return nc


_PROGRAM_CACHE = {}


def run(x, edge_index, W_rel, b_rel, W_root, W_lin, b_lin, gamma, beta, cfg):
    per_core = preprocess(x, edge_index, cfg)

    wcat = np.concatenate([W_rel.T, W_root.T, W_lin.T], axis=1)
    pcat = np.stack([b_rel, gamma, beta], axis=1)
    shared = {
        "Wcat": np.ascontiguousarray(wcat).astype(ml_dtypes.bfloat16),
        "Pcat": np.ascontiguousarray(pcat).astype(np.float32),
    }
    # b_lin is dropped: it shifts every x3 column equally, so BatchNorm's
    # mean subtraction cancels it exactly.
    in_maps = [dict(m, **shared) for m in per_core]

    key = (cfg["n_cores"], cfg["T"])
    if key not in _PROGRAM_CACHE:
        _PROGRAM_CACHE[key] = build_program(cfg)
    nc = _PROGRAM_CACHE[key]

    res = bass_utils.run_bass_kernel_spmd(
        nc, in_maps, core_ids=list(range(cfg["n_cores"])))
    n = x.shape[0]
    out = np.empty((n, 128), dtype=np.float32)
    for c in range(cfg["n_cores"]):
        nid, cols = cfg["colmap"][c]
        dev = np.asarray(res.results[c]["out"])  # [128, 4T] bf16
        out[nid] = dev[:, cols].T.astype(np.float32)
    return out


def make_cfg():
    return {
        "n_cores": N_CORES,
        "npc": NPC,
        "n_total": N_NODES,
    }


def kernel(x, edge_index, batch, W_rel, b_rel, W_root, W_lin, b_lin, gamma,
           beta):
    x = np.asarray(x, dtype=np.float32)
    cfg = make_cfg()
    return run(x, np.asarray(edge_index), np.asarray(W_rel, dtype=np.float32),
               np.asarray(b_rel, dtype=np.float32),
               np.asarray(W_root, dtype=np.float32),
               np.asarray(W_lin, dtype=np.float32),
               np.asarray(b_lin, dtype=np.float32),
               np.asarray(gamma, dtype=np.float32),
               np.asarray(beta, dtype=np.float32), cfg)

